# revision 5
# baseline (speedup 1.0000x reference)
"""LocalGaussianBlur v2 — Trainium2 Bass kernel (7x7 truncation).

Math: sigma = modulator[h,w] in (0,1); u = 1/(2 sigma^2 + 1e-8);
q = exp(-u) <= exp(-0.5).  Weight of tap (j,t) is q^(j^2+t^2).
Since q <= 0.6065, taps with |j| or |t| >= 4 carry < 5.4e-4 of the
kernel mass; normalizing by the truncated sum s3 = 1 + 2(q+q^4+q^9)
makes the truncated kernel a proper weighted average, so rel err from
truncation is ~5e-4.  m = j^2+t^2 groups kept: {1,2,4,5,8,9,10} and
m=10's partner merged as q^10 C10 ~= q^9 * (0.45 C10) (worst ~0.22%).

out = [Xc + q C1 + q^2 C2 + q^4 C4 + q^5 C5 + q^8 C8
        + q^9 (C9 + 0.45 C10)] / s3^2

Layout per core (8-way H-shard, 64 rows each):
  partitions p = rq*32+cb (4 row-quarters x 32 col-blocks),
  per-partition spatial block 16x16, X with halo rows 3 / cols 4
  -> X tile [128, 3ch, 22, 24] bf16.  A second copy XO shifted by one
  column keeps every column-pair add 4B-aligned for the DVE 2x bf16
  mode.  Weight maps are per-pixel [128,16,16], broadcast over the
  channel axis with stride-0 APs.
"""

import os
import numpy as np

H = W = 512
C = 3
NC = 8
RS = H // NC        # 64 rows per core
RQ = 4              # row-quarters per core
CB = 32             # col blocks
TR = 16             # block rows
TC = 16             # block cols
RHL = 3             # row halo
CHL = 4             # col halo (even => aligned bf16 slices)
XR = TR + 2 * RHL   # 22
XC = TC + 2 * CHL   # 24
P = 128

_NC_CACHE = {}


def _build_nc():
    if "nc" in _NC_CACHE:
        return _NC_CACHE["nc"]
    import concourse.bass as bass  # noqa: F401
    from concourse import bacc
    import concourse.mybir as mybir
    from concourse.tile import TileContext
    from concourse.ap import AP as BassAP

    f32 = mybir.dt.float32
    bf16 = mybir.dt.bfloat16
    AF = mybir.ActivationFunctionType
    ALU = mybir.AluOpType

    nc = bacc.Bacc()
    x = nc.dram_tensor("x", [P, C, XR, XC], bf16, kind="ExternalInput")
    xo = nc.dram_tensor("xo", [P, C, XR, XC], bf16, kind="ExternalInput")
    xn = nc.dram_tensor("xn", [P, C, XR, TC], bf16, kind="ExternalInput")
    md = nc.dram_tensor("md", [P, TR, TC], f32, kind="ExternalInput")
    out = nc.dram_tensor("out", [P, C, TR, TC], f32, kind="ExternalOutput")

    use_gp = os.environ.get("LGB2_GP", "0") == "1"
    nrep = int(os.environ.get("LGB2_REPEAT", "1"))

    with TileContext(nc) as tc:
        with tc.tile_pool(name="main", bufs=1) as pool:
            X = pool.tile([P, C, XR, XC], bf16, tag="X")
            XO = pool.tile([P, C, XR, XC], bf16, tag="XO")
            # XN: center cols only (no col halo) -> rows are contiguous, so
            # slot-strided multi-row-pair APs collapse to 3 free dims
            XN = pool.tile([P, C, XR, TC], bf16, tag="XN")
            MD = pool.tile([P, TR, TC], f32, tag="MD")
            nc.sync.dma_start(out=MD[:], in_=md[:])
            nc.sync.dma_start(out=X[:], in_=x[:])
            nc.sync.dma_start(out=XO[:], in_=xo[:])
            nc.sync.dma_start(out=XN[:], in_=xn[:])
            EPS = pool.tile([P, 1], f32, tag="EPS")
            nc.vector.memset(EPS[:], 1e-4)

            # all tiles allocated up-front so the compute body can sit
            # inside a hardware loop (repeat-timing mode)
            V = pool.tile([P, TR, TC], f32, tag="V")
            U = pool.tile([P, TR, TC], f32, tag="U")
            # slots: q1,q4,q9,q2,q5,q10,q8
            QAB = pool.tile([P, 7, TR, TC], bf16, tag="QAB")
            SS = pool.tile([P, TR, TC], f32, tag="SS")
            RN = pool.tile([P, TR, TC], f32, tag="RN")
            NRM = pool.tile([P, TR, TC], f32, tag="NRM")
            A = pool.tile([P, 3, C, XR, TC], bf16, tag="A")
            # CC slots: C1, C4, C9, C2, C5, C10, C8 | scratch: C5b, C10b
            CC = pool.tile([P, 9, C, TR, TC], bf16, tag="CC")
            # PR slots: P1, P4, P9, P2, P5, P10, P8
            PR = pool.tile([P, 7, C, TR, TC], bf16, tag="PR")
            L1 = pool.tile([P, 3, C, TR, TC], bf16, tag="L1")
            W1 = pool.tile([P, C, TR, TC], bf16, tag="W1")
            W2 = pool.tile([P, C, TR, TC], bf16, tag="W2")
            # F1 sums ~42% of the output mass -> bf16 rounding here is
            # <=0.17% worst-case; only F2 (adds the dominant center term)
            # and OUT stay fp32
            F1 = pool.tile([P, C, TR, TC], bf16, tag="F1")
            F2 = pool.tile([P, C, TR, TC], f32, tag="F2")
            OUTT = pool.tile([P, C, TR, TC], f32, tag="OUTT")

            def body():
                # ACT head: V = 2*sigma^2 + eps starts immediately
                nc.scalar.activation(V[:], MD[:], AF.Square,
                                     scale=float(np.sqrt(2.0)), bias=EPS[:])

                # ---- A_t: column pair sums (t-slot, ch, rows incl halo);
                # independent of the weight chain -> DVE is busy while ACT
                # squares and the u-chain result is awaited ----
                # A1 needs all 22 rows (j=3 shifts); A2 only rows 1..20,
                # A3 only rows 2..19 — skip halo rows nobody reads
                nc.vector.tensor_tensor(
                    A[:, 0], XO[:, :, :, 2:2 + TC], XO[:, :, :, 4:4 + TC],
                    ALU.add)
                nc.vector.tensor_tensor(
                    A[:, 1, :, 1:21], X[:, :, 1:21, 2:2 + TC],
                    X[:, :, 1:21, 6:6 + TC], ALU.add)
                nc.vector.tensor_tensor(
                    A[:, 2, :, 2:20], XO[:, :, 2:20, 0:TC],
                    XO[:, :, 2:20, 6:6 + TC], ALU.add)

                # ---- per-pixel u = 1/(2 sigma^2 + eps); eps folded into the
                # Square's bias ((r2*sig + 1e-4)^2 = 2 sig^2 + 1e-8 + tiny
                # cross term ~2.8e-4*sig, a ~3e-4 relative wobble on u) ----
                nc.vector.reciprocal_approx_fast(U[:], V[:])

                # ---- weight maps on ACT (overlap the DVE pair-sum stage) ----
                for i, m in enumerate((1, 4, 9, 2, 5, 10, 8)):
                    nc.scalar.activation(QAB[:, i], U[:], AF.Exp,
                                         scale=float(-m))

                # ---- X row-pairs (j=1,2,3) in ONE op: the slot axis walks
                # the row offset (stride -TC / +TC elements of the halo-free
                # XN copy, so dims collapse to 3) -> CC[0:3]; then += A
                # center rows -> CC slots (C1, C4, C9) ----
                def xslot(j0, slot_stride):
                    b = XN[:, None, :, j0:j0 + TR, :] \
                        .broadcast_to([P, 3, C, TR, TC])
                    ap2 = list(b.ap)
                    ap2[1] = [slot_stride * TC, 3]
                    return BassAP(b.tensor, b.offset, ap2)

                if os.environ.get("LGB2_XMERGE", "1") == "1":
                    nc.vector.tensor_tensor(
                        CC[:, 0:3], xslot(RHL - 1, -1), xslot(RHL + 1, 1),
                        ALU.add)
                else:
                    for i, j in enumerate((1, 2, 3)):
                        nc.vector.tensor_tensor(
                            CC[:, i],
                            XN[:, :, RHL - j:RHL - j + TR, :],
                            XN[:, :, RHL + j:RHL + j + TR, :],
                            ALU.add)
                nc.vector.tensor_tensor(
                    CC[:, 0:3], CC[:, 0:3], A[:, :, :, RHL:RHL + TR, :],
                    ALU.add)

                # ---- normalization 1/s^2 = exp(-2 ln s), s = 1+2(q1+q4+q9):
                # the recip moves to ACT (ln+exp share one table set) ----
                nc.vector.tensor_tensor(SS[:], QAB[:, 0], QAB[:, 1], ALU.add)
                nc.vector.tensor_tensor(SS[:], SS[:], QAB[:, 2], ALU.add)
                nc.scalar.activation(RN[:], SS[:], AF.Copy, bias=1.0,
                                     scale=2.0)
                nc.scalar.activation(RN[:], RN[:], AF.Ln)
                nc.scalar.activation(NRM[:], RN[:], AF.Exp, scale=-2.0)

                def arows(sl, j):
                    return (A[:, sl, :, RHL - j:RHL - j + TR, :],
                            A[:, sl, :, RHL + j:RHL + j + TR, :])

                # ---- A row-pairs ----
                lo, hi = arows(slice(0, 3), 1)
                nc.vector.tensor_tensor(CC[:, 3:6], lo, hi, ALU.add)
                # -> CC (.., C2, C5a, C10a)
                # j=2 pairs (C5b, C8) written slot-REVERSED so C8 lands at
                # CC[6] (joins the 7-slot product) and C5b at CC[7] (scratch)
                lo, hi = arows(slice(0, 2), 2)

                def rev2(apv):
                    ap2 = list(apv.ap)
                    sl = ap2[1]
                    assert sl[1] == 2
                    off = apv.offset + sl[0]
                    return BassAP(apv.tensor, off, [ap2[0], [-sl[0], 2]]
                                  + ap2[2:])

                nc.vector.tensor_tensor(rev2(CC[:, 6:8]), lo, hi, ALU.add)
                lo, hi = arows(0, 3)
                nc.vector.tensor_tensor(CC[:, 8], lo, hi, ALU.add)   # C10b
                # C5 += C5b ; C10 += C10b   (contiguous slot pair, one op)
                nc.vector.tensor_tensor(CC[:, 4:6], CC[:, 4:6], CC[:, 7:9],
                                        ALU.add)

                # ---- products ----
                def bc3(q):  # [P,TR,TC] -> [P,C,TR,TC] stride-0 channel bc
                    return q[:, None, :, :].broadcast_to([P, C, TR, TC])

                nc.vector.tensor_tensor(
                    PR[:],
                    QAB[:, :, None, :, :].broadcast_to([P, 7, C, TR, TC]),
                    CC[:, 0:7], ALU.mult)  # (P1, P4, P9, P2, P5, P10, P8)

                # ---- reduction tree (small terms in bf16, tail in fp32) ----
                nc.vector.tensor_tensor(L1[:], PR[:, 1:4], PR[:, 4:7],
                                        ALU.add)   # (P4+P2, P9+P5, P10+P8)
                nc.vector.tensor_tensor(W1[:], L1[:, 1], L1[:, 2], ALU.add)
                nc.vector.tensor_tensor(W2[:], W1[:], L1[:, 0], ALU.add)
                nc.vector.tensor_tensor(F1[:], W2[:], PR[:, 0], ALU.add)
                nc.vector.tensor_tensor(
                    F2[:], F1[:], XN[:, :, RHL:RHL + TR, :], ALU.add)
                nc.vector.tensor_tensor(OUTT[:], F2[:], bc3(NRM), ALU.mult)

            if nrep == 1:
                body()
            else:
                # 16x unrolled hw loop: the per-iteration For_i machinery
                # (~1.4 us) amortizes over 16 serial bodies in timing mode
                UN = 16
                assert nrep % UN == 0, nrep
                with tc.For_i(0, nrep // UN, 1):
                    for _ in range(UN):
                        body()
            nc.sync.dma_start(out=out[:], in_=OUTT[:])

    nc.compile()
    _NC_CACHE["nc"] = nc
    return nc


def _stage_inputs(img, modulator):
    import ml_dtypes
    x = np.ascontiguousarray(np.asarray(img, dtype=np.float32))[0]  # (3,H,W)
    mod = np.ascontiguousarray(np.asarray(modulator, dtype=np.float32))
    xpad = np.pad(x, ((0, 0), (RHL, RHL), (CHL, CHL + 1)), mode="edge")
    # (3, 518, 521)
    idx_r = (np.arange(RQ) * TR)[:, None] + np.arange(XR)[None, :]  # (4,22)
    idx_c = (np.arange(CB) * TC)[:, None] + np.arange(XC)[None, :]  # (32,24)
    idx_cn = (np.arange(CB) * TC)[:, None] + CHL + np.arange(TC)[None, :]
    mir = (np.arange(RQ) * TR)[:, None] + np.arange(TR)[None, :]
    mic = (np.arange(CB) * TC)[:, None] + np.arange(TC)[None, :]
    in_maps = []
    for core in range(NC):
        sub = xpad[:, core * RS:core * RS + RS + 2 * RHL, :]  # (3,70,521)
        # (3, 4, 32, 22, 24) -> (128, 3, 22, 24)
        blk = sub[:, idx_r[:, None, :, None], idx_c[None, :, None, :]]
        xt = np.ascontiguousarray(
            blk.transpose(1, 2, 0, 3, 4).reshape(P, C, XR, XC)
        ).astype(ml_dtypes.bfloat16)
        blk_o = sub[:, idx_r[:, None, :, None], idx_c[None, :, None, :] + 1]
        xot = np.ascontiguousarray(
            blk_o.transpose(1, 2, 0, 3, 4).reshape(P, C, XR, XC)
        ).astype(ml_dtypes.bfloat16)
        blk_n = sub[:, idx_r[:, None, :, None], idx_cn[None, :, None, :]]
        xnt = np.ascontiguousarray(
            blk_n.transpose(1, 2, 0, 3, 4).reshape(P, C, XR, TC)
        ).astype(ml_dtypes.bfloat16)
        msub = mod[core * RS:core * RS + RS, :]  # (64, 512)
        mdt = np.ascontiguousarray(
            msub[mir[:, None, :, None], mic[None, :, None, :]]
            .reshape(P, TR, TC))
        in_maps.append({"x": xt, "xo": xot, "xn": xnt, "md": mdt})
    return in_maps


def kernel(img, modulator):
    from concourse.bass_utils import run_bass_kernel_spmd

    nc = _build_nc()
    in_maps = _stage_inputs(img, modulator)
    res = run_bass_kernel_spmd(nc, in_maps, list(range(NC))).results
    # per-core out [128, 3, 16, 16] -> (3, 64, 512)
    parts = []
    for i in range(NC):
        o = np.asarray(res[i]["out"]).reshape(RQ, CB, C, TR, TC)
        parts.append(o.transpose(2, 0, 3, 1, 4).reshape(C, RS, W))
    out = np.concatenate(parts, axis=1)
    return np.ascontiguousarray(out[None], dtype=np.float32)
